# revision 3
# baseline (speedup 1.0000x reference)
"""Clustered Linformer Attention — TRN2 Bass kernel, batch-parallel, all-bf16.

Per core (one batch element b):
  A:  x^T via PE transpose; q^T = wq^T x^T ; k = x^T-stat @ wk ; v likewise (PE, bf16)
  B:  kproj/vproj accumulated IN PSUM across all 8 n-groups via M=64 matmuls
      writing disjoint 64-partition slices (one zero-region start per bank
      partition-range; second sub-chain relies on pending-zero first-touch)
  C:  scores^T_h = kpTz_h q_h^T /8 -> exp (ACT) ; out^T_h via vp2 stationary with
      a ones column extracting softmax row-sums S ; S rows gathered by tiny
      SBUF-to-SBUF DMAs, reciprocal, K=8 one-hot broadcast matmul, gpsimd
      normalize-multiply ; dense y = concat @ wd + b, single bf16 DMA per strip

Host converts all inputs to bf16 (ml_dtypes): halves HBM traffic (E/F are 67MB
of the 83MB f32 per-core stream) and removes every stationary-rounding copy the
f32r kernel needed. DMA queues split by issuing engine: x+E on SP, F+weights on
ACT HWDGE. Measured via repeat-program delta (repeat=17 vs 1, min wall over 4):
285656 ns/iter vs 532045 baseline. PE-bound: ~225us of matmul at 2.4GHz.
"""
import sys
import numpy as np

for _p in ("/opt/trn_rl_repo", "/root/.axon_site/_ro/trn_rl_repo"):
    if _p not in sys.path:
        sys.path.insert(0, _p)

import ml_dtypes
import concourse.bacc as bacc
import concourse.tile as tile
from concourse import mybir
from concourse.bass_utils import run_bass_kernel_spmd

B, N, D = 8, 4096, 512
H, R = 8, 256
DEP = D // H          # 64
P = 128
NG = 8                # n-groups for phase A/B
GN = N // NG          # 512 rows per group
NS = 8                # n-strips for phase C..G
SN = N // NS          # 512 cols per strip
F32 = mybir.dt.float32
BF16 = mybir.dt.bfloat16
EXPF = mybir.ActivationFunctionType.Exp
BF = ml_dtypes.bfloat16

_cache = {}


def build_program(repeat=1):
    key = ("nc", repeat)
    if key in _cache:
        return _cache[key]
    nc = bacc.Bacc("TRN2", target_bir_lowering=False, debug=False)
    x = nc.dram_tensor("x", [N, D], BF16, kind="ExternalInput").ap()
    wq = nc.dram_tensor("wq", [D, D], BF16, kind="ExternalInput").ap()
    wk = nc.dram_tensor("wk", [D, D], BF16, kind="ExternalInput").ap()
    wv = nc.dram_tensor("wv", [D, D], BF16, kind="ExternalInput").ap()
    wd = nc.dram_tensor("wd", [D, D], BF16, kind="ExternalInput").ap()
    E = nc.dram_tensor("E", [H, N, R], BF16, kind="ExternalInput").ap()
    Fm = nc.dram_tensor("F", [H, N, R], BF16, kind="ExternalInput").ap()
    ident_in = nc.dram_tensor("ident", [P, P], BF16, kind="ExternalInput").ap()
    hb8_in = nc.dram_tensor("hb8", [8, D], BF16, kind="ExternalInput").ap()
    ones_in = nc.dram_tensor("ones", [P, 1], BF16, kind="ExternalInput").ap()
    bbc_in = nc.dram_tensor("b_bc", [P, D], F32, kind="ExternalInput").ap()
    y = nc.dram_tensor("y", [N, D], BF16, kind="ExternalOutput").ap()

    with tile.TileContext(nc) as tc, nc.allow_low_precision(reason="bf16 kernel"):
      for _rep in range(repeat):
        with tc.tile_pool(name="outer", bufs=1) as po:
            # ---- persistent tiles ----
            qT = [po.tile([P, N], BF16, tag=f"qT{c}", name=f"qT{c}") for c in range(4)]
            kpTz = [po.tile([P, R], BF16, tag=f"kpTz{h}", name=f"kpTz{h}") for h in range(H)]
            vp2 = [[po.tile([P, P], BF16, tag=f"vp2_{h}_{rc}", name=f"vp2_{h}_{rc}")
                    for rc in range(2)] for h in range(H)]
            wd_t = [po.tile([P, D], BF16, tag=f"wd{c}", name=f"wd{c}") for c in range(4)]
            hb8 = po.tile([8, D], BF16, tag="hb8", name="hb8")
            b_bc = po.tile([P, D], F32, tag="b_bc", name="b_bc")
            ident = po.tile([P, P], BF16, tag="ident", name="ident")
            ones_t = po.tile([P, 1], BF16, tag="ones", name="ones")

            nc.sync.dma_start(ident[:], ident_in)
            nc.scalar.dma_start(hb8[:], hb8_in)
            nc.sync.dma_start(b_bc[:], bbc_in)
            nc.scalar.dma_start(ones_t[:], ones_in)
            for c in range(4):
                nc.scalar.dma_start(wd_t[c][:], wd[c * P:(c + 1) * P, :])
            # zero-init: kpTz off-parity rows; vp2 zero cols (+ ones col below)
            for h in range(H):
                nc.gpsimd.memset(kpTz[h][:], 0.0)
                for rc in range(2):
                    nc.gpsimd.memset(vp2[h][rc][:], 0.0)
            for h in range(H):
                oro = DEP * (1 - h % 2)
                for rc in range(2):
                    nc.vector.tensor_copy(vp2[h][rc][:, oro:oro + 1], ones_t[:])

            # ================= PHASE A+B =================
            with tc.tile_pool(name="pw", bufs=1) as pw, \
                 tc.tile_pool(name="pa", bufs=8) as pa, \
                 tc.tile_pool(name="pkv", bufs=8) as pkv, \
                 tc.tile_pool(name="pef", bufs=8) as pef, \
                 tc.tile_pool(name="psM", bufs=4, space="PSUM") as psM, \
                 tc.tile_pool(name="psA", bufs=1, space="PSUM") as psA:
                psT = psM
                psQ = psM

                # prefetch group-0 x before the weight loads on the SP queue
                xg0_t = []
                for i in range(4):
                    t = pa.tile([P, D], BF16, tag="xg", name="xg")
                    nc.sync.dma_start(t[:], x[i * P:(i + 1) * P, :])
                    xg0_t.append(t)
                wq_t = [pw.tile([P, D], BF16, tag=f"wq{c}", name=f"wq{c}") for c in range(4)]
                wk_t = [pw.tile([P, D], BF16, tag=f"wk{c}", name=f"wk{c}") for c in range(4)]
                wv_t = [pw.tile([P, D], BF16, tag=f"wv{c}", name=f"wv{c}") for c in range(4)]
                for c in range(4):
                    nc.sync.dma_start(wq_t[c][:], wq[c * P:(c + 1) * P, :])
                    nc.scalar.dma_start(wk_t[c][:], wk[c * P:(c + 1) * P, :])
                    nc.gpsimd.dma_start(wv_t[c][:], wv[c * P:(c + 1) * P, :])

                # PSUM-resident projection accumulators: kpP[j] rows 0:64 =
                # head 2(2j)+par.. layout: [128 part = dep-pair, 2 (pidx in
                # pair), R]; accumulated over all 8 groups.
                kpP = [psA.tile([P, 2, R], F32, tag=f"kpP{j}", name=f"kpP{j}")
                       for j in range(2)]
                vpP = [psA.tile([P, 2, R], F32, tag=f"vpP{j}", name=f"vpP{j}")
                       for j in range(2)]
                # explicit zero + start=False accumulation everywhere: makes
                # the 4 sub-chains per bank order-independent (a lone
                # start=True marks the whole 2KB zero-region pending, which
                # breaks commutativity if the scheduler reorders chains)
                for j in range(2):
                    nc.vector.memset(kpP[j][:], 0.0)
                    nc.vector.memset(vpP[j][:], 0.0)

                for g in range(NG):
                    n0 = g * GN
                    if g == 0:
                        xg_t = xg0_t
                    else:
                        xg_t = []
                        for i in range(4):
                            t = pa.tile([P, D], BF16, tag="xg", name="xg")
                            nc.sync.dma_start(t[:], x[n0 + i * P:n0 + (i + 1) * P, :])
                            xg_t.append(t)
                    xT_t = [pa.tile([P, GN], BF16, tag="xT", name="xT") for c in range(4)]
                    for c in range(4):
                        tpb = psT.tile([P, 4, P], BF16, tag="mix", name="tp")
                        for i in range(4):
                            nc.tensor.transpose(
                                tpb[:, i, :], xg_t[i][:, c * P:(c + 1) * P], ident[:])
                        nc.vector.tensor_copy(
                            xT_t[c][:].rearrange("p (i q) -> p i q", i=4), tpb[:])
                    # q^T
                    for dq in range(4):
                        qp = psQ.tile([P, GN], F32, tag="mix", name="qp")
                        for c in range(4):
                            nc.tensor.matmul(
                                qp[:], wq_t[c][:, dq * P:(dq + 1) * P], xT_t[c][:],
                                start=(c == 0), stop=(c == 3))
                        nc.scalar.copy(qT[dq][:, n0:n0 + GN], qp[:])
                    # k, v
                    kg_t = [pkv.tile([P, D], BF16, tag="kg", name="kg") for i in range(4)]
                    vg_t = [pkv.tile([P, D], BF16, tag="vg", name="vg") for i in range(4)]
                    for i in range(4):
                        kp_ = psQ.tile([P, D], F32, tag="mix", name="qp")
                        for c in range(4):
                            nc.tensor.matmul(
                                kp_[:], xT_t[c][:, i * P:(i + 1) * P], wk_t[c][:],
                                start=(c == 0), stop=(c == 3))
                        nc.scalar.copy(kg_t[i][:], kp_[:])
                        vp_ = psQ.tile([P, D], F32, tag="mix", name="qp")
                        for c in range(4):
                            nc.tensor.matmul(
                                vp_[:], xT_t[c][:, i * P:(i + 1) * P], wv_t[c][:],
                                start=(c == 0), stop=(c == 3))
                        nc.vector.tensor_copy(vg_t[i][:], vp_[:])
                    # B: project k, v through E_h, F_h; M=64 matmuls write the
                    # head's own 64-partition slice, accumulating in PSUM
                    # across groups.
                    for pidx in range(4):
                        j, sub = divmod(pidx, 2)
                        for par in range(2):
                            h = 2 * pidx + par
                            ro = DEP * par
                            ksl = slice(pidx * P + ro, pidx * P + ro + DEP)
                            Eh = pef.tile([P, 4, R], BF16, tag="ef", name="ef")
                            nc.sync.dma_start(
                                Eh[:], E[h, n0:n0 + GN, :].rearrange(
                                    "(i p) r -> p i r", p=P))
                            Fh = pef.tile([P, 4, R], BF16, tag="ef", name="ef")
                            nc.scalar.dma_start(
                                Fh[:], Fm[h, n0:n0 + GN, :].rearrange(
                                    "(i p) r -> p i r", p=P))
                            for i in range(4):
                                nc.tensor.matmul(
                                    kpP[j][ro:ro + DEP, sub, :],
                                    kg_t[i][:, ksl], Eh[:, i, :],
                                    start=False,
                                    stop=(g == NG - 1 and i == 3),
                                    skip_group_check=True)
                            for i in range(4):
                                nc.tensor.matmul(
                                    vpP[j][ro:ro + DEP, sub, :],
                                    vg_t[i][:, ksl], Fh[:, i, :],
                                    start=False,
                                    stop=(g == NG - 1 and i == 3),
                                    skip_group_check=True)

                # evict kproj into zero-padded per-head stationary tiles;
                # transpose vproj pairs to natural layout
                for pidx in range(4):
                    j, sub = divmod(pidx, 2)
                    for par in range(2):
                        h = 2 * pidx + par
                        ro = DEP * par
                        nc.vector.tensor_copy(
                            kpTz[h][ro:ro + DEP, :], kpP[j][ro:ro + DEP, sub, :])
                    vpS = pa.tile([P, R], BF16, tag="vpS", name="vpS")
                    nc.vector.tensor_copy(vpS[:], vpP[j][:, sub, :])
                    for rc in range(2):
                        vt = psT.tile([P, P], BF16, tag="mix", name="vt")
                        nc.tensor.transpose(
                            vt[:], vpS[:, rc * P:(rc + 1) * P], ident[:])
                        for par in range(2):
                            h = 2 * pidx + par
                            ro = DEP * par
                            nc.vector.tensor_copy(
                                vp2[h][rc][:, ro:ro + DEP], vt[:, ro:ro + DEP])

            # ================= PHASE C..G =================
            with tc.tile_pool(name="pexp", bufs=8) as pexp, \
                 tc.tile_pool(name="pstag", bufs=10) as pstag, \
                 tc.tile_pool(name="pcs", bufs=2) as pcs, \
                 tc.tile_pool(name="pbc", bufs=6) as pbc, \
                 tc.tile_pool(name="psml", bufs=4) as psml, \
                 tc.tile_pool(name="ps3", bufs=3, space="PSUM") as ps3, \
                 tc.tile_pool(name="ps2", bufs=2, space="PSUM") as ps2, \
                 tc.tile_pool(name="ps1", bufs=1, space="PSUM") as ps1:
                for s in range(NS):
                    c0 = s * SN
                    csR = pcs.tile([P, 4, SN], BF16, tag="csR", name="csR")
                    S_t = psml.tile([8, SN], BF16, tag="S", name="S")
                    stags = []
                    for h in range(H):
                        c = h // 2
                        oro = DEP * (1 - h % 2)
                        expT_t = [pexp.tile([P, SN], BF16, tag="expT", name="expT")
                                  for rc in range(2)]
                        for rc in range(2):
                            scp = ps3.tile([P, SN], F32, tag="sc", name="sc")
                            nc.tensor.matmul(
                                scp[:],
                                kpTz[h][:, rc * P:(rc + 1) * P],
                                qT[c][:, c0:c0 + SN],
                                start=True, stop=True)
                            nc.scalar.activation(
                                expT_t[rc][:], scp[:], EXPF,
                                scale=float(1.0 / np.sqrt(np.float32(DEP))))
                        fop = ps2.tile([P, SN], F32, tag="fo", name="fo")
                        for rc in range(2):
                            nc.tensor.matmul(
                                fop[:], vp2[h][rc][:], expT_t[rc][:],
                                start=(rc == 0), stop=(rc == 1))
                        stag = pstag.tile([P, SN], BF16, tag="stag", name="stag")
                        nc.vector.tensor_copy(stag[:], fop[:])
                        nc.sync.dma_start(S_t[h:h + 1, :], stag[oro:oro + 1, :])
                        stags.append(stag)
                    Sr8 = psml.tile([8, SN], BF16, tag="Sr", name="Sr")
                    nc.vector.reciprocal(Sr8[:], S_t[:])
                    for p in range(4):
                        bcp = ps1.tile([P, SN], F32, tag="bc", name="bc")
                        nc.tensor.matmul(bcp[:], hb8[:, p * P:(p + 1) * P], Sr8[:],
                                         start=True, stop=True)
                        bcs = pbc.tile([P, SN], F32, tag="bcs", name="bcs")
                        if p % 2 == 0:
                            nc.scalar.copy(bcs[:], bcp[:])
                        else:
                            nc.vector.tensor_copy(bcs[:], bcp[:])
                        for par in range(2):
                            h = 2 * p + par
                            ro = DEP * par
                            nc.gpsimd.tensor_mul(
                                csR[ro:ro + DEP, p, :],
                                stags[h][ro:ro + DEP, :],
                                bcs[ro:ro + DEP, :])
                    ys = psml.tile([P, 4, D], BF16, tag="ysb", name="ysb")
                    for jj in range(4):
                        yp = ps2.tile([P, D], F32, tag="yp", name="yp")
                        for c in range(4):
                            nc.tensor.matmul(
                                yp[:], csR[:, c, jj * P:(jj + 1) * P], wd_t[c][:],
                                start=(c == 0), stop=(c == 3))
                        nc.vector.tensor_add(ys[:, jj, :], yp[:], b_bc[:])
                    nc.sync.dma_start(
                        y[c0:c0 + SN, :].rearrange("(j p) d -> p j d", p=P), ys[:])

    nc.compile()
    _cache[key] = nc
    return nc


def make_in_maps(x, wq, wk, wv, E, F, w_dense, b_dense):
    bf = lambda a: np.ascontiguousarray(np.asarray(a, np.float32).astype(BF))
    x = bf(x)
    hb8 = np.zeros((8, D), dtype=BF)
    for p in range(4):
        for m in range(P):
            hb8[2 * p + m // DEP, p * P + m] = 1.0
    consts = {
        "wq": bf(wq), "wk": bf(wk), "wv": bf(wv), "wd": bf(w_dense),
        "E": bf(E), "F": bf(F),
        "ident": np.eye(P, dtype=BF),
        "hb8": hb8,
        "ones": np.ones((P, 1), dtype=BF),
        "b_bc": np.tile(np.asarray(b_dense, np.float32).reshape(1, D), (P, 1)),
    }
    return [{"x": x[b], **consts} for b in range(B)]


def kernel(x, wq, wk, wv, E, F, w_dense, b_dense):
    nc = build_program()
    in_maps = make_in_maps(x, wq, wk, wv, E, F, w_dense, b_dense)
    res = run_bass_kernel_spmd(nc, in_maps, list(range(B)))
    out = np.stack([np.asarray(res.results[b]["y"]) for b in range(B)], axis=0)
    return out.astype(np.float32)


# revision 4
# speedup vs baseline: 1074.2223x; 1074.2223x over previous
"""Clustered Linformer Attention — TRN2 Bass kernel, batch-parallel, all-bf16.

Per core (one batch element b):
  A:  x^T via PE transpose; q^T = wq^T x^T ; k = x^T-stat @ wk ; v likewise (PE, bf16)
  B:  kproj/vproj accumulated IN PSUM across all 8 n-groups via M=64 matmuls
      writing disjoint 64-partition slices (one zero-region start per bank
      partition-range; second sub-chain relies on pending-zero first-touch)
  C:  scores^T_h = kpTz_h q_h^T /8 -> exp (ACT) ; out^T_h via vp2 stationary with
      a ones column extracting softmax row-sums S ; S rows gathered by tiny
      SBUF-to-SBUF DMAs, reciprocal, K=8 one-hot broadcast matmul, gpsimd
      normalize-multiply ; dense y = concat @ wd + b, single bf16 DMA per strip

Host converts all inputs to bf16 (ml_dtypes): halves HBM traffic (E/F are 67MB
of the 83MB f32 per-core stream) and removes every stationary-rounding copy the
f32r kernel needed. DMA queues split by issuing engine: x+E on SP, F+weights on
ACT HWDGE. Measured via repeat-program delta (repeat=17 vs 1, min wall):
358381 ns/iter vs 532045 baseline. PE-bound: ~225us of matmul at 2.4GHz.
"""
import sys
import numpy as np

for _p in ("/opt/trn_rl_repo", "/root/.axon_site/_ro/trn_rl_repo"):
    if _p not in sys.path:
        sys.path.insert(0, _p)

import ml_dtypes
import concourse.bacc as bacc
import concourse.tile as tile
from concourse import mybir
from concourse.bass_utils import run_bass_kernel_spmd

B, N, D = 8, 4096, 512
H, R = 8, 256
DEP = D // H          # 64
P = 128
NG = 8                # n-groups for phase A/B
GN = N // NG          # 512 rows per group
NS = 8                # n-strips for phase C..G
SN = N // NS          # 512 cols per strip
F32 = mybir.dt.float32
BF16 = mybir.dt.bfloat16
EXPF = mybir.ActivationFunctionType.Exp
BF = ml_dtypes.bfloat16

_cache = {}


def build_program(repeat=1):
    key = ("nc", repeat)
    if key in _cache:
        return _cache[key]
    nc = bacc.Bacc("TRN2", target_bir_lowering=False, debug=False)
    x = nc.dram_tensor("x", [N, D], BF16, kind="ExternalInput").ap()
    wq = nc.dram_tensor("wq", [D, D], BF16, kind="ExternalInput").ap()
    wk = nc.dram_tensor("wk", [D, D], BF16, kind="ExternalInput").ap()
    wv = nc.dram_tensor("wv", [D, D], BF16, kind="ExternalInput").ap()
    wd = nc.dram_tensor("wd", [D, D], BF16, kind="ExternalInput").ap()
    E = nc.dram_tensor("E", [H, N, R], BF16, kind="ExternalInput").ap()
    Fm = nc.dram_tensor("F", [H, N, R], BF16, kind="ExternalInput").ap()
    ident_in = nc.dram_tensor("ident", [P, P], BF16, kind="ExternalInput").ap()
    hb8_in = nc.dram_tensor("hb8", [8, D], BF16, kind="ExternalInput").ap()
    ones_in = nc.dram_tensor("ones", [P, 1], BF16, kind="ExternalInput").ap()
    bbc_in = nc.dram_tensor("b_bc", [P, D], F32, kind="ExternalInput").ap()
    y = nc.dram_tensor("y", [N, D], BF16, kind="ExternalOutput").ap()

    with tile.TileContext(nc) as tc, nc.allow_low_precision(reason="bf16 kernel"):
      for _rep in range(repeat):
        with tc.tile_pool(name="outer", bufs=1) as po:
            # ---- persistent tiles ----
            qT = [po.tile([P, N], BF16, tag=f"qT{c}", name=f"qT{c}") for c in range(4)]
            kpTz = [po.tile([P, R], BF16, tag=f"kpTz{h}", name=f"kpTz{h}") for h in range(H)]
            vp2 = [[po.tile([P, P], BF16, tag=f"vp2_{h}_{rc}", name=f"vp2_{h}_{rc}")
                    for rc in range(2)] for h in range(H)]
            wd_t = [po.tile([P, D], BF16, tag=f"wd{c}", name=f"wd{c}") for c in range(4)]
            hb8 = po.tile([8, D], BF16, tag="hb8", name="hb8")
            b_bc = po.tile([P, D], F32, tag="b_bc", name="b_bc")
            ident = po.tile([P, P], BF16, tag="ident", name="ident")
            ones_t = po.tile([P, 1], BF16, tag="ones", name="ones")

            nc.sync.dma_start(ident[:], ident_in)
            nc.scalar.dma_start(hb8[:], hb8_in)
            nc.sync.dma_start(b_bc[:], bbc_in)
            nc.scalar.dma_start(ones_t[:], ones_in)
            for c in range(4):
                nc.scalar.dma_start(wd_t[c][:], wd[c * P:(c + 1) * P, :])
            # zero-init: kpTz off-parity rows; vp2 zero cols (+ ones col below)
            for h in range(H):
                nc.gpsimd.memset(kpTz[h][:], 0.0)
                for rc in range(2):
                    nc.gpsimd.memset(vp2[h][rc][:], 0.0)
            for h in range(H):
                oro = DEP * (1 - h % 2)
                for rc in range(2):
                    nc.vector.tensor_copy(vp2[h][rc][:, oro:oro + 1], ones_t[:])

            # ================= PHASE A+B =================
            with tc.tile_pool(name="pw", bufs=1) as pw, \
                 tc.tile_pool(name="pa", bufs=8) as pa, \
                 tc.tile_pool(name="pkv", bufs=8) as pkv, \
                 tc.tile_pool(name="pef", bufs=8) as pef, \
                 tc.tile_pool(name="psM", bufs=4, space="PSUM") as psM, \
                 tc.tile_pool(name="psA", bufs=1, space="PSUM") as psA:
                psT = psM
                psQ = psM

                # prefetch group-0 x before the weight loads on the SP queue
                xg0_t = []
                for i in range(4):
                    t = pa.tile([P, D], BF16, tag="xg", name="xg")
                    nc.sync.dma_start(t[:], x[i * P:(i + 1) * P, :])
                    xg0_t.append(t)
                wq_t = [pw.tile([P, D], BF16, tag=f"wq{c}", name=f"wq{c}") for c in range(4)]
                wk_t = [pw.tile([P, D], BF16, tag=f"wk{c}", name=f"wk{c}") for c in range(4)]
                wv_t = [pw.tile([P, D], BF16, tag=f"wv{c}", name=f"wv{c}") for c in range(4)]
                for c in range(4):
                    nc.sync.dma_start(wq_t[c][:], wq[c * P:(c + 1) * P, :])
                    nc.scalar.dma_start(wk_t[c][:], wk[c * P:(c + 1) * P, :])
                    nc.gpsimd.dma_start(wv_t[c][:], wv[c * P:(c + 1) * P, :])

                # PSUM-resident projection accumulators: kpP[j] rows 0:64 =
                # head 2(2j)+par.. layout: [128 part = dep-pair, 2 (pidx in
                # pair), R]; accumulated over all 8 groups.
                kpP = [psA.tile([P, 2, R], F32, tag=f"kpP{j}", name=f"kpP{j}")
                       for j in range(2)]
                vpP = [psA.tile([P, 2, R], F32, tag=f"vpP{j}", name=f"vpP{j}")
                       for j in range(2)]
                # explicit zero + start=False accumulation everywhere: makes
                # the 4 sub-chains per bank order-independent (a lone
                # start=True marks the whole 2KB zero-region pending, which
                # breaks commutativity if the scheduler reorders chains)
                for j in range(2):
                    nc.vector.memset(kpP[j][:], 0.0)
                    nc.vector.memset(vpP[j][:], 0.0)

                for g in range(NG):
                    n0 = g * GN
                    if g == 0:
                        xg_t = xg0_t
                    else:
                        xg_t = []
                        for i in range(4):
                            t = pa.tile([P, D], BF16, tag="xg", name="xg")
                            nc.sync.dma_start(t[:], x[n0 + i * P:n0 + (i + 1) * P, :])
                            xg_t.append(t)
                    xT_t = [pa.tile([P, GN], BF16, tag="xT", name="xT") for c in range(4)]
                    for c in range(4):
                        tpb = psT.tile([P, 4, P], BF16, tag="mix", name="tp")
                        for i in range(4):
                            nc.tensor.transpose(
                                tpb[:, i, :], xg_t[i][:, c * P:(c + 1) * P], ident[:])
                        nc.vector.tensor_copy(
                            xT_t[c][:].rearrange("p (i q) -> p i q", i=4), tpb[:])
                    # q^T
                    for dq in range(4):
                        qp = psQ.tile([P, GN], F32, tag="mix", name="qp")
                        for c in range(4):
                            nc.tensor.matmul(
                                qp[:], wq_t[c][:, dq * P:(dq + 1) * P], xT_t[c][:],
                                start=(c == 0), stop=(c == 3))
                        nc.scalar.copy(qT[dq][:, n0:n0 + GN], qp[:])
                    # k, v
                    kg_t = [pkv.tile([P, D], BF16, tag="kg", name="kg") for i in range(4)]
                    vg_t = [pkv.tile([P, D], BF16, tag="vg", name="vg") for i in range(4)]
                    for i in range(4):
                        kp_ = psQ.tile([P, D], F32, tag="mix", name="qp")
                        for c in range(4):
                            nc.tensor.matmul(
                                kp_[:], xT_t[c][:, i * P:(i + 1) * P], wk_t[c][:],
                                start=(c == 0), stop=(c == 3))
                        nc.scalar.copy(kg_t[i][:], kp_[:])
                        vp_ = psQ.tile([P, D], F32, tag="mix", name="qp")
                        for c in range(4):
                            nc.tensor.matmul(
                                vp_[:], xT_t[c][:, i * P:(i + 1) * P], wv_t[c][:],
                                start=(c == 0), stop=(c == 3))
                        nc.vector.tensor_copy(vg_t[i][:], vp_[:])
                    # B: project k, v through E_h, F_h; M=64 matmuls write the
                    # head's own 64-partition slice, accumulating in PSUM
                    # across groups.
                    for pidx in range(4):
                        j, sub = divmod(pidx, 2)
                        for par in range(2):
                            h = 2 * pidx + par
                            ro = DEP * par
                            ksl = slice(pidx * P + ro, pidx * P + ro + DEP)
                            Eh = pef.tile([P, 4, R], BF16, tag="ef", name="ef")
                            nc.sync.dma_start(
                                Eh[:], E[h, n0:n0 + GN, :].rearrange(
                                    "(i p) r -> p i r", p=P))
                            Fh = pef.tile([P, 4, R], BF16, tag="ef", name="ef")
                            nc.scalar.dma_start(
                                Fh[:], Fm[h, n0:n0 + GN, :].rearrange(
                                    "(i p) r -> p i r", p=P))
                            for i in range(4):
                                nc.tensor.matmul(
                                    kpP[j][ro:ro + DEP, sub, :],
                                    kg_t[i][:, ksl], Eh[:, i, :],
                                    start=False,
                                    stop=(g == NG - 1 and i == 3),
                                    skip_group_check=True)
                            for i in range(4):
                                nc.tensor.matmul(
                                    vpP[j][ro:ro + DEP, sub, :],
                                    vg_t[i][:, ksl], Fh[:, i, :],
                                    start=False,
                                    stop=(g == NG - 1 and i == 3),
                                    skip_group_check=True)

                # evict kproj into zero-padded per-head stationary tiles;
                # transpose vproj pairs to natural layout
                for pidx in range(4):
                    j, sub = divmod(pidx, 2)
                    for par in range(2):
                        h = 2 * pidx + par
                        ro = DEP * par
                        nc.vector.tensor_copy(
                            kpTz[h][ro:ro + DEP, :], kpP[j][ro:ro + DEP, sub, :])
                    vpS = pa.tile([P, R], BF16, tag="vpS", name="vpS")
                    nc.vector.tensor_copy(vpS[:], vpP[j][:, sub, :])
                    for rc in range(2):
                        vt = psT.tile([P, P], BF16, tag="mix", name="vt")
                        nc.tensor.transpose(
                            vt[:], vpS[:, rc * P:(rc + 1) * P], ident[:])
                        for par in range(2):
                            h = 2 * pidx + par
                            ro = DEP * par
                            nc.vector.tensor_copy(
                                vp2[h][rc][:, ro:ro + DEP], vt[:, ro:ro + DEP])

            # ================= PHASE C..G =================
            with tc.tile_pool(name="pexp", bufs=8) as pexp, \
                 tc.tile_pool(name="pstag", bufs=10) as pstag, \
                 tc.tile_pool(name="pcs", bufs=2) as pcs, \
                 tc.tile_pool(name="pbc", bufs=6) as pbc, \
                 tc.tile_pool(name="psml", bufs=4) as psml, \
                 tc.tile_pool(name="ps3", bufs=3, space="PSUM") as ps3, \
                 tc.tile_pool(name="ps2", bufs=2, space="PSUM") as ps2, \
                 tc.tile_pool(name="ps1", bufs=1, space="PSUM") as ps1:
                for s in range(NS):
                    c0 = s * SN
                    csR = pcs.tile([P, 4, SN], BF16, tag="csR", name="csR")
                    S_t = psml.tile([8, SN], BF16, tag="S", name="S")
                    stags = []
                    for h in range(H):
                        c = h // 2
                        oro = DEP * (1 - h % 2)
                        expT_t = [pexp.tile([P, SN], BF16, tag="expT", name="expT")
                                  for rc in range(2)]
                        for rc in range(2):
                            scp = ps3.tile([P, SN], F32, tag="sc", name="sc")
                            nc.tensor.matmul(
                                scp[:],
                                kpTz[h][:, rc * P:(rc + 1) * P],
                                qT[c][:, c0:c0 + SN],
                                start=True, stop=True)
                            nc.scalar.activation(
                                expT_t[rc][:], scp[:], EXPF,
                                scale=float(1.0 / np.sqrt(np.float32(DEP))))
                        fop = ps2.tile([P, SN], F32, tag="fo", name="fo")
                        for rc in range(2):
                            nc.tensor.matmul(
                                fop[:], vp2[h][rc][:], expT_t[rc][:],
                                start=(rc == 0), stop=(rc == 1))
                        stag = pstag.tile([P, SN], BF16, tag="stag", name="stag")
                        nc.vector.tensor_copy(stag[:], fop[:])
                        nc.sync.dma_start(S_t[h:h + 1, :], stag[oro:oro + 1, :])
                        stags.append(stag)
                    Sr8 = psml.tile([8, SN], BF16, tag="Sr", name="Sr")
                    nc.vector.reciprocal(Sr8[:], S_t[:])
                    for p in range(4):
                        bcp = ps1.tile([P, SN], F32, tag="bc", name="bc")
                        nc.tensor.matmul(bcp[:], hb8[:, p * P:(p + 1) * P], Sr8[:],
                                         start=True, stop=True)
                        bcs = pbc.tile([P, SN], F32, tag="bcs", name="bcs")
                        if p % 2 == 0:
                            nc.scalar.copy(bcs[:], bcp[:])
                        else:
                            nc.vector.tensor_copy(bcs[:], bcp[:])
                        for par in range(2):
                            h = 2 * p + par
                            ro = DEP * par
                            nc.gpsimd.tensor_mul(
                                csR[ro:ro + DEP, p, :],
                                stags[h][ro:ro + DEP, :],
                                bcs[ro:ro + DEP, :])
                    ys = psml.tile([P, 4, D], BF16, tag="ysb", name="ysb")
                    for jj in range(4):
                        yp = ps2.tile([P, D], F32, tag="yp", name="yp")
                        for c in range(4):
                            nc.tensor.matmul(
                                yp[:], csR[:, c, jj * P:(jj + 1) * P], wd_t[c][:],
                                start=(c == 0), stop=(c == 3))
                        nc.vector.tensor_add(ys[:, jj, :], yp[:], b_bc[:])
                    nc.sync.dma_start(
                        y[c0:c0 + SN, :].rearrange("(j p) d -> p j d", p=P), ys[:])

    nc.compile()
    _cache[key] = nc
    return nc


def make_in_maps(x, wq, wk, wv, E, F, w_dense, b_dense):
    bf = lambda a: np.ascontiguousarray(np.asarray(a, np.float32).astype(BF))
    x = bf(x)
    hb8 = np.zeros((8, D), dtype=BF)
    for p in range(4):
        for m in range(P):
            hb8[2 * p + m // DEP, p * P + m] = 1.0
    consts = {
        "wq": bf(wq), "wk": bf(wk), "wv": bf(wv), "wd": bf(w_dense),
        "E": bf(E), "F": bf(F),
        "ident": np.eye(P, dtype=BF),
        "hb8": hb8,
        "ones": np.ones((P, 1), dtype=BF),
        "b_bc": np.tile(np.asarray(b_dense, np.float32).reshape(1, D), (P, 1)),
    }
    return [{"x": x[b], **consts} for b in range(B)]


def kernel(x, wq, wk, wv, E, F, w_dense, b_dense):
    nc = build_program()
    in_maps = make_in_maps(x, wq, wk, wv, E, F, w_dense, b_dense)
    res = run_bass_kernel_spmd(nc, in_maps, list(range(B)))
    out = np.stack([np.asarray(res.results[b]["y"]) for b in range(B)], axis=0)
    return out.astype(np.float32)
